# revision 18
# baseline (speedup 1.0000x reference)
"""Cross-conditional GPT2 sparse attention block on 8 Trainium2 NeuronCores.

Sharding: core = (batch b in 0..3) x (head-group g in 0..1, 6 heads each).
Each core computes, for its (b, g):
  qT/kT = (Wq_g @ x_b^T + bq_g)  laid out [d_on_partitions, L]
  v     = x_b @ Wv_g^T + bv_g    natural layout [L, 384], interleaved with a
          ones column per head ([L, 6, 65]) so att@v also yields the softmax
          denominator for free.
  scores are computed *transposed* (sT[j, i]) so that softmax needs no
  transpose at all: host-built additive masks (-30000 on masked entries)
  are accumulated into the score PSUM by identity-weight PE matmuls, exp
  on ACT underflows them to 0, att@v via lhsT=v (natural layout),
  denominator broadcast across partitions via a K=1 PE matmul, then the
  partial output projection with Wp[:, g]^T.
Host sums the two per-batch partials and adds bp.

v2 performance structure (TRN2 PE p-states: the PE only reaches 2.4 GHz
after ~3us of gap-free execution; any stall drops it to 1.2 GHz):
  - all inputs are host-pre-swizzled so every DMA is contiguous per
    partition (128 fat descriptors/tensor instead of ~per-row thin ones;
    descriptor issue rate, not bandwidth, dominated the old 22us lead-in)
  - phase A loops i-chunk-outer so compute starts once chunk 0 lands
  - the attention phase is software-pipelined: a flat work-queue of score
    chunks is emitted with LOOK items of lookahead, so the in-order PE
    queue always has an independent score matmul to run while the
    exp (ACT) -> mask (DVE) -> att@v chain of an earlier chunk drains.
    Output-projection chains are interleaved into the following group's
    stream as additional PE filler; softmax finalize is split in two
    stages (cast on Pool, then bcast/recip/mult) two slots apart.
"""

import sys

sys.path.insert(0, "/opt/trn_rl_repo")

from contextlib import ExitStack

import ml_dtypes
import numpy as np

import concourse.bacc as bacc
import concourse.bass as bass
import concourse.mybir as mybir
import concourse.tile as tile
from concourse.bass_utils import run_bass_kernel_spmd

# ---- problem constants (hardcoded per spec) ----
B = 4
T = 512
N = 8
C = 768
NHEAD = 12
L = 3 * T + 4 * N  # 1568
P = 128
G = C // 2  # 384 channels per head-group
NH = 6  # heads per core
D = 64  # head dim
ET = C // P  # 6 e-tiles (contraction of x @ W)
CT = G // P  # 3 c-tiles of the group's channels
NJT = (L + P - 1) // P  # 13 j tiles (12x128 + 32)
I_CHUNKS = [(0, 512), (512, 512), (1024, 512), (1536, 32)]
SCALE = 1.0 / 8.0  # 1/sqrt(64)
LOOK = 2  # attention pipeline lookahead (pair items in flight ahead of att@v)

F32 = mybir.dt.float32
BF16 = mybir.dt.bfloat16
F16 = mybir.dt.float16

_NC = None  # cached compiled Bass program


def _jl(jt):
    return P if jt < NJT - 1 else L - (NJT - 1) * P  # 128 or 32


def _ich_of(a):
    return 3 if a == 1536 else a // 512


# (group) -> per-jt score interval (a, ln).
# g0 = upper rows (i 0..512), jts 0..3; g1 = lower rows; g2 = torso+text rows.
def _grp_interval(g, jt):
    j0 = jt * P
    f0 = (jt % 4) * P if jt <= 11 else 0
    if g == 0:
        return (j0, 512 - j0) if jt <= 3 else None
    if g == 1:
        s = j0 if jt <= 3 else f0
        return (512 + s, 512 - s)
    s = j0 if jt <= 3 else f0
    return (1024 + s, 544 - s)


# mask kind per (group, jt): 'T1' | 'T2' | 'TXT' | None
def _grp_mask(g, jt):
    if jt == 12:
        return "TXT" if g in (1, 2) else None
    if g == 0:
        return "T1"
    if g == 1:
        return "T1" if jt <= 3 else "T2"
    return "T1" if jt <= 7 else "T2"


_GRP_ITS = {0: range(0, 4), 1: range(4, 8), 2: range(8, 13)}


def _grp_jts(g):
    return [jt for jt in range(NJT) if _grp_interval(g, jt) is not None]


def _chunks_of(g, jt):
    a, ln = _grp_interval(g, jt)
    out = [(a, min(ln, 512))]
    if ln > 512:
        out.append((a + 512, ln - 512))
    return out


def _head_pairs(g):
    """Pack the head's score chunks two per 2-bank PSUM tile (chunk 2 at
    column offset 512) so ONE exp instruction covers both. Big chunks are
    pair leaders: exp padding waste is 512-cl(leader), so leaders are the
    top half by width. Returns pair items [[(jt,ca,cl,parts),..1-2]];
    parts carry per-ich att@v stop flags for the scrambled order."""
    chunks = []
    for jt in _grp_jts(g):
        for ca, cl in _chunks_of(g, jt):
            chunks.append((jt, ca, cl))
    chunks.sort(key=lambda c: -c[2])
    nlead = (len(chunks) + 1) // 2
    pairs = []
    for i in range(nlead):
        pair = [chunks[i]]
        if nlead + i < len(chunks):
            pair.append(chunks[nlead + i])
        pairs.append(pair)
    flat = [c for pair in pairs for c in pair]
    parts_of = []
    last = {}
    for ci, (jt, ca, cl) in enumerate(flat):
        parts = [(ca, cl, 0)]
        if ca < 1536 < ca + cl:
            parts = [(ca, 1536 - ca, 0), (1536, ca + cl - 1536, 1536 - ca)]
        pl = []
        for pa, plen, poff in parts:
            ich = _ich_of(pa)
            last[ich] = (ci, len(pl))
            pl.append([pa, plen, poff, ich, False])
        parts_of.append(pl)
    for ich, (ci, pi) in last.items():
        parts_of[ci][pi][4] = True  # stop flag on the last write per ich
    out = []
    k = 0
    for pair in pairs:
        rec = []
        for c in pair:
            rec.append((c[0], c[1], c[2], parts_of[k]))
            k += 1
        out.append(rec)
    return out


def _attn_items():
    """Flat attention work queue. 'pair' items carry 1-2 score chunks
    sharing one PSUM tile + exp; 'oproj' items are output-projection
    chains for the PREVIOUS group, interleaved as PE filler."""
    items = []
    for g in range(3):
        hp = _head_pairs(g)
        pair_items = []
        for h in range(NH):
            for pi, pair in enumerate(hp):
                pair_items.append(("pair", g, h, pair, pi == len(hp) - 1))
        if g > 0:
            op = [
                ("oproj", g - 1, it, nch)
                for it in _GRP_ITS[g - 1]
                for nch in (0, 1)
            ]
            merged = []
            oi = 0
            for i, ci in enumerate(pair_items):
                merged.append(ci)
                if oi < len(op) and i % 2 == 1:
                    merged.append(op[oi])
                    oi += 1
            merged += op[oi:]
            pair_items = merged
        items += pair_items
    items += [("oproj", 2, it, nch) for it in _GRP_ITS[2] for nch in (0, 1)]
    return items


def _build_program():
    nc = bacc.Bacc("TRN2", target_bir_lowering=False, debug=False)

    xa_d = nc.dram_tensor("xa", [P, 3, ET, 512], F16, kind="ExternalInput")
    xb_d = nc.dram_tensor("xb", [P, ET, 32], F16, kind="ExternalInput")
    wq_d = nc.dram_tensor("wqS", [P, ET, G], F16, kind="ExternalInput")
    wk_d = nc.dram_tensor("wkS", [P, ET, G], F16, kind="ExternalInput")
    wv_d = nc.dram_tensor("wvS", [P, ET, G], F16, kind="ExternalInput")
    wp_d = nc.dram_tensor("wpS", [P, CT, C], F16, kind="ExternalInput")
    bq_d = nc.dram_tensor("bqP", [P, CT], F32, kind="ExternalInput")
    bk_d = nc.dram_tensor("bkP", [P, CT], F32, kind="ExternalInput")
    bv_d = nc.dram_tensor("bvB", [P, G], F32, kind="ExternalInput")
    maskd_d = nc.dram_tensor("maskD", [P, 2, P], F16, kind="ExternalInput")
    maskt_d = nc.dram_tensor("maskTxt", [32, 1024], F16, kind="ExternalInput")
    ident_d = nc.dram_tensor("identP", [P, P], F16, kind="ExternalInput")
    out_d = nc.dram_tensor("out_part", [L, C], F32, kind="ExternalOutput")

    with tile.TileContext(nc) as tc, ExitStack() as big:
        persist = big.enter_context(tc.tile_pool(name="persist", bufs=1))

        # persistent SBUF tensors
        qT = persist.tile([P, CT, L], F16, name="qT")
        kT = persist.tile([P, CT, L], F16, name="kT")
        v_ones = persist.tile([P, NJT, NH, D + 1], F16, name="v_ones")
        maskD = persist.tile([P, 2, P], F16, name="maskD_sb")
        maskTx = persist.tile([32, 1024], F16, name="maskTx_sb")
        identP = persist.tile([P, P], F16, name="identP_sb")
        yT = persist.tile([P, CT, L], F16, name="yT")
        wp_sb = persist.tile([P, CT, C], F16, name="wp_sb")
        ones64 = persist.tile([1, D], F16, name="ones64")
        bv_sb = persist.tile([P, G], F32, name="bv_sb")

        nc.gpsimd.memset(ones64[:], 1.0)
        nc.gpsimd.memset(v_ones[:], 1.0)

        # ---------- Phase A: projections ----------
        with (
            tc.tile_pool(name="phA", bufs=1) as phA,
            tc.tile_pool(name="psA", bufs=2, space="PSUM") as psA,
        ):
            xTa = phA.tile([P, 3, ET, 512], F16, name="xTa_sb")
            xTb = phA.tile([P, ET, 32], F16, name="xTb_sb")
            wq_sb = phA.tile([P, ET, G], F16, name="wq_sb")
            wk_sb = phA.tile([P, ET, G], F16, name="wk_sb")
            wv_sb = phA.tile([P, ET, G], F16, name="wv_sb")
            bq_sb = phA.tile([P, CT], F32, name="bq_sb")
            bk_sb = phA.tile([P, CT], F32, name="bk_sb")

            # priority-ordered contiguous DMAs (128 descriptors each)
            nc.sync.dma_start(wq_sb[:], wq_d[:])
            nc.sync.dma_start(bq_sb[:], bq_d[:])
            nc.sync.dma_start(wk_sb[:], wk_d[:])
            nc.sync.dma_start(bk_sb[:], bk_d[:])
            for cchunk in range(3):
                nc.sync.dma_start(xTa[:, cchunk], xa_d[:, cchunk])
            nc.sync.dma_start(xTb[:], xb_d[:])
            nc.sync.dma_start(wv_sb[:], wv_d[:])
            nc.sync.dma_start(bv_sb[:], bv_d[:])
            nc.sync.dma_start(maskD[:], maskd_d[:])
            nc.sync.dma_start(maskTx[:], maskt_d[:])
            nc.sync.dma_start(identP[:], ident_d[:])
            nc.sync.dma_start(wp_sb[:], wp_d[:])

            def xslice(ic, et, lo, ln):
                # xT columns [i0+lo : i0+lo+ln] of e-tile et for i-chunk ic
                if ic < 3:
                    return xTa[:, ic, et, lo : lo + ln]
                return xTb[:, et, lo : lo + ln]

            # qT / kT: out[c_tile, i] accumulated over e tiles; i-chunk
            # outer so compute starts as soon as x chunk 0 arrives.
            for ic, (i0, ilen) in enumerate(I_CHUNKS):
                for dst, w_sb, b_sb in ((qT, wq_sb, bq_sb), (kT, wk_sb, bk_sb)):
                    for ct in range(CT):
                        ps = psA.tile([P, 512], F32, name="ps_qk", tag="ps_qk")
                        for et in range(ET):
                            nc.tensor.matmul(
                                ps[:, :ilen],
                                w_sb[:, et, ct * P : (ct + 1) * P],
                                xslice(ic, et, 0, ilen),
                                start=(et == 0),
                                stop=(et == ET - 1),
                            )
                        nc.vector.tensor_scalar(
                            dst[:, ct, i0 : i0 + ilen],
                            ps[:, :ilen],
                            b_sb[:, ct : ct + 1],
                            None,
                            mybir.AluOpType.add,
                        )

            # v natural layout [i, 384] + bias, into the 65-strided f16 buffer
            for it in range(NJT):
                il = _jl(it)
                ic, lo = (it // 4, (it % 4) * P) if it < 12 else (3, 0)
                ps = psA.tile([P, G], F32, name="ps_v", tag="ps_v")
                for et in range(ET):
                    nc.tensor.matmul(
                        ps[:il, :],
                        xslice(ic, et, lo, il),
                        wv_sb[:, et, :],
                        start=(et == 0),
                        stop=(et == ET - 1),
                    )
                nc.vector.tensor_tensor(
                    v_ones[:il, it, :, 0:D],
                    ps[:il, :].rearrange("p (h d) -> p h d", h=NH),
                    bv_sb[:il, :].rearrange("p (h d) -> p h d", h=NH),
                    mybir.AluOpType.add,
                )

        # ---------- Phase B+C: pipelined attention + interleaved out-proj ----
        with (
            tc.tile_pool(name="phB", bufs=1) as phB,
            tc.tile_pool(name="phC", bufs=3) as phC,
            tc.tile_pool(name="psS", bufs=2, space="PSUM") as psS,
            tc.tile_pool(name="psO", bufs=1, space="PSUM") as psO,
            tc.tile_pool(name="psY", bufs=2, space="PSUM") as psY,
            tc.tile_pool(name="psY3", bufs=1, space="PSUM") as psY3,
        ):
            items = _attn_items()
            nitems = len(items)
            # per-(g,h) attention state
            st = {}
            for g in range(3):
                for h in range(NH):
                    st[(g, h)] = {"psy": {}, "started": set(), "pt": {}}
            osb = {}
            y3 = []  # lazily-allocated shared text-column accumulator
            deferred = []  # fin args emitted this slot
            fin_due = []  # [(slot_recorded, fin-args), ...]

            def emit_pair(g, h, pair):
                """Stage 1: score matmuls for 1-2 chunks into one 2-bank
                PSUM tile (chunk 2 at column 512), masks accumulated on the
                PE via identity-weight matmuls, then ONE exp for the pair."""
                s = st[(g, h)]
                pof = D * (h % 2)
                ct = h // 2
                ps_p = psS.tile([P, 1024], F32, name="ps_p", tag="ps_p")
                width = 0
                maxjl = 0
                for k, (jt, ca, cl, parts) in enumerate(pair):
                    jl = _jl(jt)
                    maxjl = max(maxjl, jl)
                    off = 512 * k
                    a, _ = _grp_interval(g, jt)
                    mk = _grp_mask(g, jt)
                    masked = ca == a and mk is not None
                    nc.tensor.matmul(
                        ps_p[:jl, off : off + cl],
                        kT[pof : pof + D, ct, jt * P : jt * P + jl],
                        qT[pof : pof + D, ct, ca : ca + cl],
                        start=True,
                        stop=not masked,
                        skip_group_check=True,
                    )
                    if masked and mk in ("T1", "T2"):
                        nc.tensor.matmul(
                            ps_p[:jl, off : off + P],
                            identP[:, :],
                            maskD[:, 0 if mk == "T1" else 1, :],
                            start=False,
                            stop=True,
                            skip_group_check=True,
                        )
                    elif masked:  # TXT
                        m0 = ca - 512
                        nc.tensor.matmul(
                            ps_p[:jl, off : off + cl],
                            identP[0:32, 0:32],
                            maskTx[:32, m0 : m0 + cl],
                            start=False,
                            stop=True,
                            skip_group_check=True,
                        )
                    width = off + cl
                pt = phB.tile([P, 1024], F16, name="pT", tag="pT", bufs=8)
                nc.scalar.activation(
                    pt[:maxjl, :width],
                    ps_p[:maxjl, :width],
                    mybir.ActivationFunctionType.Exp,
                    bias=0.0,
                    scale=SCALE,
                )
                for k, (jt, ca, cl, parts) in enumerate(pair):
                    s["pt"][(jt, ca)] = (pt, 512 * k)

            def emit_attv(g, h, jt, ca, cl, parts):
                s = st[(g, h)]
                jl = _jl(jt)
                pt, off = s["pt"].pop((jt, ca))
                for pa, pl, poff, ich, stop in parts:
                    if ich == 3:
                        if not y3:
                            y3.append(
                                psY3.tile([D + 1, 192], F32, name="ps_y3t", tag="y3")
                            )
                        s["psy"].setdefault(3, (y3[0], h * 32))
                    elif ich not in s["psy"]:
                        s["psy"][ich] = (
                            psY.tile([D + 1, 512], F32, name="ps_yg", tag="ps_y"),
                            0,
                        )
                    psy, base = s["psy"][ich]
                    ob = base + pa - (0, 512, 1024, 1536)[ich]
                    nc.tensor.matmul(
                        psy[:, ob : ob + pl],
                        v_ones[:jl, jt, h, :],
                        pt[:jl, off + poff : off + poff + pl],
                        start=ich not in s["started"],
                        stop=stop,
                        skip_group_check=True,
                    )
                    s["started"].add(ich)

            def emit_fin1(g, h):
                """Stage 1 of softmax finalize: denominator row to SBUF f16."""
                s = st[(g, h)]
                for ich, (psy, base) in s["psy"].items():
                    ilen = I_CHUNKS[ich][1]
                    den = phB.tile([1, 512], F16, name="den", tag="den", bufs=4)
                    nc.vector.tensor_copy(
                        den[0:1, :ilen], psy[D : D + 1, base : base + ilen]
                    )
                    deferred.append((g, h, ich, den))

            def emit_fin2(g, h, ich, den):
                """Stage 2: broadcast den across partitions (K=1 PE matmul),
                reciprocal, multiply into yT."""
                pof = D * (h % 2)
                ct = h // 2
                psy, base = st[(g, h)]["psy"][ich]
                i0, ilen = I_CHUNKS[ich]
                ps_bc = psO.tile([P, 512], F32, name="ps_bc", tag="ps_o")
                nc.tensor.matmul(
                    ps_bc[:D, :ilen],
                    ones64[0:1, :],
                    den[0:1, :ilen],
                    start=True,
                    stop=True,
                )
                rc = phB.tile([D, 512], F32, name="rc", tag="rc", bufs=4)
                nc.vector.reciprocal_approx_fast(out=rc[:, :ilen], in_=ps_bc[:D, :ilen])
                nc.vector.tensor_tensor(
                    yT[pof : pof + D, ct, i0 : i0 + ilen],
                    psy[0:D, base : base + ilen],
                    rc[:, :ilen],
                    mybir.AluOpType.mult,
                )

            def drain_fins_for_group(g):
                keep = []
                for slot, args in fin_due:
                    if args[0] == g:
                        emit_fin2(*args)
                    else:
                        keep.append((slot, args))
                fin_due[:] = keep

            def emit_oproj(g, it, nch):
                drain_fins_for_group(g)
                il = _jl(it)
                if nch == 0:
                    osb[it] = phC.tile([P, C], F32, name="o_sb", tag="o_sb")
                o_sb = osb[it]
                ps_o = psO.tile([P, 512], F32, name="ps_o", tag="ps_o")
                for kt in range(CT):
                    nc.tensor.matmul(
                        ps_o[:il, :G],
                        yT[:, kt, it * P : it * P + il],
                        wp_sb[:, kt, nch * G : (nch + 1) * G],
                        start=(kt == 0),
                        stop=(kt == CT - 1),
                        skip_group_check=True,
                    )
                nc.any.tensor_copy(o_sb[:il, nch * G : (nch + 1) * G], ps_o[:il, :G])
                if nch == 1:
                    nc.sync.dma_start(out_d[it * P : it * P + il, :], o_sb[:il, :])
                    del osb[it]

            for i in range(nitems + LOOK):
                if i < nitems:
                    itm = items[i]
                    if itm[0] == "pair":
                        emit_pair(itm[1], itm[2], itm[3])
                if i >= LOOK:
                    # release fins one slot after recording (their DVE cast
                    # has had a full slot to finish, so the PE bcast won't
                    # stall right after the last att@v)
                    while fin_due and fin_due[0][0] <= i - 1:
                        _, args = fin_due.pop(0)
                        emit_fin2(*args)
                    itm = items[i - LOOK]
                    if itm[0] == "pair":
                        _, g, h, pair, final = itm
                        for jt, ca, cl, parts in pair:
                            emit_attv(g, h, jt, ca, cl, parts)
                        if final:
                            emit_fin1(g, h)
                        while deferred:
                            fin_due.append((i, deferred.pop(0)))
                    else:
                        emit_oproj(itm[1], itm[2], itm[3])
            # drain any remaining fins
            while fin_due:
                _, args = fin_due.pop(0)
                emit_fin2(*args)

    nc.compile()
    return nc


def _build_mask_np(seg_starts, seg_ends):
    """True = masked. Mirrors reference._build_mask in numpy."""
    ML = 3 * T
    tril = np.tril(np.ones((T, T), dtype=bool))
    sl = np.tril(np.ones((T, T), dtype=bool), -1)
    m = np.zeros((L, L), dtype=bool)
    m[:ML, :ML] = True
    m[0:T, 0:T] = ~tril
    m[T : 2 * T, 0:T] = ~tril
    m[T : 2 * T, T : 2 * T] = ~sl
    m[T : 2 * T, 2 * T : 3 * T] = ~sl
    m[2 * T : 3 * T, 0:T] = ~tril
    m[2 * T : 3 * T, T : 2 * T] = ~tril
    m[2 * T : 3 * T, 2 * T : 3 * T] = ~sl
    m[:ML, ML:] = True
    frames = np.arange(T)[None, :, None]
    allowed = (frames >= seg_starts[:, None, :]) & (frames < seg_ends[:, None, :])
    mask = np.broadcast_to(m[None], (B, L, L)).copy()
    for row0, col_blocks in ((T, (0, 2, 3)), (2 * T, (1, 2, 3))):
        for j in col_blocks:
            c0 = ML + j * N
            mask[:, row0 : row0 + T, c0 : c0 + N] &= ~allowed
    return mask


def get_nc():
    global _NC
    if _NC is None:
        _NC = _build_program()
    return _NC


def _swz(mat, nt):
    """[nt*128, X] -> [128, nt, X] contiguous f16 (partition-major)."""
    x = np.asarray(mat)
    return np.ascontiguousarray(
        x.reshape(nt, P, x.shape[1]).transpose(1, 0, 2)
    ).astype(np.float16)


def make_in_maps(x, Wq, bq, Wk, bk, Wv, bv, Wp, bp, seg_starts, seg_ends):
    mask = _build_mask_np(np.asarray(seg_starts), np.asarray(seg_ends))
    r = np.arange(P)
    NEG = np.float16(-30000.0)  # exp((s-30000)/8) underflows to exactly 0
    maskDh = np.zeros((P, 2, P), dtype=np.float16)
    maskDh[:, 0, :] = np.where(r[:, None] <= r[None, :], np.float16(0), NEG)
    maskDh[:, 1, :] = np.where(r[:, None] < r[None, :], np.float16(0), NEG)
    identP = np.eye(P, dtype=np.float16)
    in_maps = []
    for core in range(8):
        b, g = core // 2, core % 2
        gs = slice(g * G, (g + 1) * G)
        allowT = ~mask[b].T  # [j, i]
        maskTx = np.ascontiguousarray(
            np.where(allowT[1536:1568, 512:1536], np.float16(0), NEG)
        )
        xsw = _swz(x[b].T, ET)  # [P, ET, L]
        xa = np.ascontiguousarray(
            np.stack([xsw[:, :, c * 512 : (c + 1) * 512] for c in range(3)], axis=1)
        )  # [P, 3, ET, 512]
        xb_ = np.ascontiguousarray(xsw[:, :, 1536:1568])  # [P, ET, 32]
        in_maps.append(
            {
                "xa": xa,
                "xb": xb_,
                "wqS": _swz(Wq[gs, :].T, ET),
                "wkS": _swz(Wk[gs, :].T, ET),
                "wvS": _swz(Wv[gs, :].T, ET),
                "wpS": _swz(Wp[:, gs].T, CT),
                "bqP": np.ascontiguousarray(bq[gs].reshape(CT, P).T),
                "bkP": np.ascontiguousarray(bk[gs].reshape(CT, P).T),
                "bvB": np.broadcast_to(bv[gs], (P, G)).copy(),
                "maskD": maskDh,
                "maskTxt": maskTx,
                "identP": identP,
            }
        )
    return in_maps


def kernel(x, Wq, bq, Wk, bk, Wv, bv, Wp, bp, seg_starts, seg_ends, T_motion=None,
           N=None, _trace=False, **_unused):
    x = np.asarray(x, np.float32)
    args = [np.asarray(a, np.float32) for a in (Wq, bq, Wk, bk, Wv, bv, Wp, bp)]
    Wq, bq, Wk, bk, Wv, bv, Wp, bp = args
    nc = get_nc()
    in_maps = make_in_maps(x, Wq, bq, Wk, bk, Wv, bv, Wp, bp, seg_starts, seg_ends)
    res = run_bass_kernel_spmd(nc, in_maps, core_ids=list(range(8)), trace=_trace)
    parts = [r["out_part"] for r in res.results]
    y = np.empty((B, L, C), np.float32)
    for b in range(B):
        y[b] = parts[2 * b] + parts[2 * b + 1] + bp
    if _trace:
        kernel.last_results = res
    return y
